# revision 1
# baseline (speedup 1.0000x reference)
"""Causal single-head attention (B=4, T=4096, D=1024, D_H=64) on 8 TRN2 cores.

Two SPMD Bass kernels with a host exchange between them. Work split per
batch b over cores (2b, 2b+1); core 2b+h owns rows [h*2048, (h+1)*2048):

Kernel 1 (all core-local):
  - Fused QKV projection: x 128x128 chunks are the stationary operand, the
    concatenated [Wq|Wk|Wv] [128, 192] block streams through, accumulating
    [t,h]-natural Q/K/V in PSUM over the 8 d-chunks.
  - Q/K chunks are PE-transposed to [h, t] layout for the score matmuls;
    V stays natural and gets a ones column appended (the softmax
    denominator then falls out of the AV matmul for free).
  - The local 2048x2048 causal triangle: S^T = K_chunk @ Q^T on PE (keys on
    partitions), exp on ACT (the 1/32 scale is folded into the activation),
    triangular-block mask multiply on DVE for diagonal chunks, then
    O[q,65] += P_chunk^T @ V' with the P chunk stationary (65-wide moving
    operand). Diagonal chunks only compute their causally valid suffix.
  - Exports the natural-layout QKV stripes for kernel 2.

Kernel 2 (after a host reshuffle): the dense rows 2048..4095 x
keys [c%2 *1024 ..+1024) rectangle of batch b. Same S/exp/AV pipeline.

Host combines: rows 0..2047 from core 2b's triangle; rows 2048..4095 =
(tri-bot + rect-left + rect-right numerators)/(summed denominators).
Plain exp without max-subtraction is safe: |scores| <~ 0.5.
"""

import numpy as np
import ml_dtypes

import concourse.bass as bass
import concourse.tile as tile
import concourse.mybir as mybir
from concourse.bass import ts
from concourse.bass_utils import run_bass_kernel_spmd

BF16_NP = ml_dtypes.bfloat16
BF16 = mybir.dt.bfloat16
FP32 = mybir.dt.float32

B, T, D, DH = 4, 4096, 1024, 64
HALF = T // 2
NCORES = 8
SCALE = float(D) ** -0.5  # 1/32, applied inside the exp activation
GROUP = 3                 # S chunks exp'd per ACT instruction (3 PSUM banks)
TRI_NKC = [4, 8, 12, 16]  # 128-key chunks per 512-query block (causal)

# tuning knobs (read by build_k1/build_k2)
CFG = {
    "xb3_early": False,    # prefetch stripe-3 x at the top
    "kt_shift_pool": False,
    "qt_copy_dve": True,
}


# ---------------------------------------------------------------------------
# Workaround: this walrus build rejects instructions carrying more than one
# sync wait ("Too many sync wait commands" in setupSyncWait). Tile's
# add_semaphores stage attaches up to ~3 waits per instruction. Post-pass:
# hoist all but the last wait of every instruction into preceding same-engine
# single-wait NoOps (engines execute their stream in order, so this is
# semantically identical).
# ---------------------------------------------------------------------------
def _split_sync_waits(nc):
    for fn in nc.m.functions:
        for bb in fn.blocks:
            insts = list(bb.instructions)
            out, ctr = [], 0
            for inst in insts:
                si = inst.sync_info
                waits = list(si.on_wait) if (si is not None and si.on_wait) else []
                if len(waits) > 1:
                    for w in waits[:-1]:
                        nop = mybir.InstNoOp(
                            name=f"{inst.name}__swait{ctr}",
                            engine=inst.engine,
                            ins=[],
                            outs=[],
                            sync_info=mybir.SyncInfo(on_wait=[w], on_update=[]),
                        )
                        out.append(nop)
                        ctr += 1
                    inst.sync_info = mybir.SyncInfo(
                        on_wait=[waits[-1]],
                        on_update=list(si.on_update or []),
                    )
                out.append(inst)
            if ctr:
                bb.instructions = out


def _tri_groups(nkc):
    out, kc0 = [], 0
    while kc0 < nkc:
        g = min(GROUP, nkc - kc0)
        out.append((kc0, g))
        kc0 += g
    return out


class CausalEmitter:
    """Emits S -> exp -> (mask) -> AV chunk groups, the AV matmuls lagging one
    exp group behind so PE always has S work queued while ACT runs.

    S^T [k=128, q<=512] is computed with K^T chunks stationary; AV uses the
    P chunk [k=128, q=128] as the stationary operand and V' [128, 65]
    moving, accumulating O[q, 65] naturally per 128-query sub-block into a
    single-bank [128, 4, 65] PSUM accumulator.

    If diag, the last 4 chunks of query block qb are diagonal: chunk with
    diag offset j covers keys [128j, 128j+128) relative to the block, so
    only the column suffix [128j:512] is computed, sub-blocks qi < j are
    skipped, and the triangular 128x128 head block gets a mask multiply.
    """

    def __init__(self, nc, vp_of, mask_sb, out_dram, spsum, opsum, ppool,
                 osb, diag, auto_flush=True):
        self.nc = nc
        self.vp_of = vp_of
        self.mask_sb = mask_sb
        self.out_dram = out_dram
        self.spsum = spsum
        self.opsum = opsum
        self.ppool = ppool
        self.osb = osb
        self.diag = diag
        self.auto_flush = auto_flush
        self.pend = []

    def _flush_one(self):
        nc = self.nc
        o_ps, p_sb, entries, nkc, qb = self.pend.pop(0)
        done = False
        for i, kc, off in entries:
            qi_min = off // 128
            for qi in range(qi_min, 4):
                if self.diag:
                    stop = kc == nkc - 4 + qi
                else:
                    stop = kc == nkc - 1
                # start=True clears the has_written bits of the WHOLE bank,
                # so only the very first AV matmul of this accumulator may
                # set it; later sub-blocks overwrite via their cleared bits.
                nc.tensor.matmul(
                    o_ps[:, qi, :],
                    lhsT=p_sb[:, i, ts(qi, 128)],
                    rhs=self.vp_of(kc),
                    start=(kc == 0 and qi == qi_min),
                    stop=stop,
                )
            done = kc == nkc - 1
        if done:
            o_sb = self.osb.tile([128, 4, 65], FP32, tag="o_sb")
            nc.vector.tensor_copy(out=o_sb, in_=o_ps)
            nc.sync.dma_start(out=self.out_dram[qb], in_=o_sb)

    def emit_qb(self, qb, nkc, qt_tile, kt_of):
        nc = self.nc
        o_ps = self.opsum.tile([128, 4, 65], FP32, tag="o_ps")
        for kc0, gsz in _tri_groups(nkc):
            s_ps = self.spsum.tile([128, GROUP, 512], FP32, tag="s_ps")
            entries = []
            for i in range(gsz):
                kc = kc0 + i
                j = kc - (nkc - 4) if self.diag else -1
                off = 128 * j if j > 0 else 0
                entries.append((i, kc, off))
                nc.tensor.matmul(
                    s_ps[:, i, off:],
                    lhsT=kt_of(kc),
                    rhs=qt_tile[:, off:512],
                    start=True,
                    stop=True,
                )
            p_sb = self.ppool.tile([128, GROUP, 512], BF16, tag="p_sb")
            nc.scalar.activation(
                out=p_sb[:, :gsz, :],
                in_=s_ps[:, :gsz, :],
                func=mybir.ActivationFunctionType.Exp,
                scale=SCALE,
            )
            if self.diag:
                for i, kc, off in entries:
                    j = kc - (nkc - 4)
                    if j >= 0:
                        moff = 128 * j
                        nc.vector.tensor_mul(
                            out=p_sb[:, i, moff : moff + 128],
                            in0=p_sb[:, i, moff : moff + 128],
                            in1=self.mask_sb,
                        )
            self.pend.append((o_ps, p_sb, entries, nkc, qb))
            if self.auto_flush and len(self.pend) > 1:
                self._flush_one()

    def finish(self):
        while self.pend:
            self._flush_one()


# ---------------------------------------------------------------------------
# Kernel 1: fused QKV projection + local causal triangle.
# ---------------------------------------------------------------------------
def build_k1():
    nc = bass.Bass()
    # xT[tb, p, dc, t] = x_shard[tb*512 + t, dc*128 + p]
    xT = nc.dram_tensor("xT", [4, 128, 8, 512], BF16, kind="ExternalInput")
    # W[p, dc, jh] = [Wq|Wk|Wv][dc*128 + p, jh]  (jh = 3*64 concatenated)
    W = nc.dram_tensor("W", [128, 8, 192], BF16, kind="ExternalInput")
    # wx0 = [W chunk0 | x stripe0 dc0 | x stripe0 dc1] packed for a 1-DMA head
    wx0 = nc.dram_tensor("wx0", [128, 1216], BF16, kind="ExternalInput")
    # mask1[k, q] = 1.0 if q >= k else 0.0
    mask1 = nc.dram_tensor("mask1", [128, 128], BF16, kind="ExternalInput")
    # exports: per stripe t, Q^T/K^T [64, 512] and V' chunks [128, 4, 65]
    qt_out = nc.dram_tensor("qt_out", [4, 64, 512], BF16, kind="ExternalOutput")
    kt_out = nc.dram_tensor("kt_out", [4, 64, 512], BF16, kind="ExternalOutput")
    vp_out = nc.dram_tensor("vp_out", [4, 128, 4, 65], BF16,
                            kind="ExternalOutput")
    # out_tri[qb, p, qi, m]: numerator cols 0..63, denominator col 64 for
    # query row qb*512 + qi*128 + p
    out_tri = nc.dram_tensor("out_tri", [4, 128, 4, 65], FP32,
                             kind="ExternalOutput")

    with tile.TileContext(nc) as tc:
        with (
            tc.tile_pool(name="const", bufs=1) as const,
            tc.tile_pool(name="xpool", bufs=3) as xpool,
            tc.tile_pool(name="ppool", bufs=7) as ppool,
            tc.tile_pool(name="osb", bufs=2) as osb,
            tc.tile_pool(name="spsum", bufs=2, space="PSUM") as spsum,
            tc.tile_pool(name="opsum", bufs=2, space="PSUM") as opsum,
        ):

            wx0_sb = const.tile([128, 1216], BF16, tag="wx0")
            nc.sync.dma_start(out=wx0_sb, in_=wx0[:])
            w0 = wx0_sb[:, 0:192]
            w1 = const.tile([128, 192], BF16, tag="w1")
            wr = const.tile([128, 6, 192], BF16, tag="wr")

            mask_sb = const.tile([128, 128], BF16, tag="mask1")
            qt_t, kt_t, vp_t = [], [], []
            emitter = CausalEmitter(
                nc,
                lambda kc: vp_t[kc // 4][:, kc % 4, :],
                mask_sb, out_tri, spsum, opsum, ppool, osb, diag=True,
                auto_flush=False,
            )

            # x prefetch: stripe 0 in 2-d-chunk pieces (fast head), stripes
            # 1-2 issued immediately behind it, stripe 3 after stripe 1's
            # projection (xbig ring of 3).
            x0_t = [wx0_sb[:, 192:704], wx0_sb[:, 704:1216]]
            nc.sync.dma_start(out=w1, in_=W[:, 1])
            for i in range(1, 4):
                xp = xpool.tile([128, 2, 512], BF16, tag=f"x0_{i}")
                nc.sync.dma_start(out=xp, in_=xT[0, :, ts(i, 2)])
                if i == 1:
                    nc.sync.dma_start(out=wr, in_=W[:, 2:8])
                    nc.sync.dma_start(out=mask_sb, in_=mask1[:])
                x0_t.extend([xp[:, 0], xp[:, 1]])
            w_t = [w0, w1] + [wr[:, dc - 2] for dc in range(2, 8)]
            xbig = {}
            pre = (1, 2, 3) if CFG["xb3_early"] else (1, 2)
            for tt in pre:
                xs = xpool.tile([128, 8, 512], BF16, tag="xbig")
                nc.sync.dma_start(out=xs, in_=xT[tt])
                xbig[tt] = xs

            # Software pipeline across 512-row stripes: project stripe t,
            # flush the previous stripe's AV matmuls (their exps ran during
            # this projection), then emit stripe t's S/exp groups.
            for t in range(4):
                if t > 0 and t not in xbig:
                    xs = xpool.tile([128, 8, 512], BF16, tag="xbig")
                    nc.sync.dma_start(out=xs, in_=xT[t])
                    xbig[t] = xs
                x_t = x0_t if t == 0 else [xbig[t][:, dc] for dc in range(8)]

                # qt holds [Q^T; K^T-junk] raw from the merged projection --
                # safe as the S moving operand because kt's rows 64..127 are
                # zeroed. kt gets K^T realigned to partitions 0..63 via an
                # SBUF->SBUF DMA (engines cannot shift partitions; DMA can).
                qt = const.tile([128, 512], BF16, tag=f"q{t}")
                kt = const.tile([128, 512], BF16, tag=f"k{t}")
                vp = const.tile([128, 4, 65], BF16, tag=f"v{t}")
                nc.vector.memset(kt[64:128, :], 0.0)
                nc.vector.memset(vp[:, :, 64:65], 1.0)
                qt_t.append(qt)
                kt_t.append(kt)
                vp_t.append(vp)

                # one 3-bank PSUM slot per stripe: bank0 = Q^T acc,
                # bank1 = K^T acc, bank2 = 4x V chunk accs. Q first and its
                # copy emitted immediately so the first S matmul of this
                # stripe isn't gated on the K/V matmuls.
                # merged QK projection: stationary [Wq|Wk] [128, 128] gives
                # PSUM rows 0..63 = Q^T, rows 64..127 = K^T in one stream
                slot = spsum.tile([128, GROUP, 512], FP32, tag="s_ps")
                for dc in range(8):
                    nc.tensor.matmul(
                        slot[:, 0, :], lhsT=w_t[dc][:, 0:128],
                        rhs=x_t[dc], start=(dc == 0), stop=(dc == 7),
                    )
                if CFG["qt_copy_dve"]:
                    nc.vector.tensor_copy(out=qt, in_=slot[:, 0, :])
                else:
                    nc.scalar.copy(out=qt, in_=slot[:, 0, :])
                if t == 0:
                    # stripe 0's first S is on the kernel head's critical
                    # path: project K directly (PE is DMA-stalled anyway)
                    # instead of waiting for copy -> realign-DMA
                    for dc in range(8):
                        nc.tensor.matmul(
                            slot[:64, 1, :], lhsT=w_t[dc][:, 64:128],
                            rhs=x_t[dc], start=(dc == 0), stop=(dc == 7),
                        )
                    nc.scalar.copy(out=kt[0:64, :], in_=slot[:64, 1, :])
                else:
                    # K^T realign to partitions 0..63 (engines cannot shift
                    # partitions; DMA can) -- hides behind the next stripe
                    if CFG["kt_shift_pool"]:
                        nc.gpsimd.dma_start(out=kt[0:64, :], in_=qt[64:128, :])
                    else:
                        nc.sync.dma_start(out=kt[0:64, :], in_=qt[64:128, :])
                for dc in range(8):
                    for tc_ in range(4):
                        nc.tensor.matmul(
                            slot[:, 2, ts(tc_, 64)],
                            lhsT=x_t[dc][:, ts(tc_, 128)],
                            rhs=w_t[dc][:, 128:192],
                            start=(dc == 0 and tc_ == 0),
                            stop=(dc == 7 and tc_ == 3),
                        )
                nc.sync.dma_start(out=qt_out[t], in_=qt[0:64, :])
                nc.sync.dma_start(out=kt_out[t], in_=qt[64:128, :])
                nc.vector.tensor_copy(
                    out=vp[:, :, 0:64],
                    in_=slot[:, 2, 0:256].rearrange("p (c h) -> p c h", c=4),
                )
                nc.sync.dma_start(out=vp_out[t], in_=vp)

                if t >= 1:
                    emitter.finish()  # AVs of tri(t-1), after proj(t) MMs
                if t == 3:
                    # no next projection to hide behind: flush with lag 1
                    emitter.auto_flush = True
                emitter.emit_qb(
                    t, TRI_NKC[t], qt_t[t],
                    lambda kc: kt_t[kc // 4][:, ts(kc % 4, 128)],
                )
            emitter.finish()
    _split_sync_waits(nc)
    return nc


# ---------------------------------------------------------------------------
# Kernel 2: dense 2048q x 1024k rectangle.
# ---------------------------------------------------------------------------
def build_k2():
    nc = bass.Bass()
    qt = nc.dram_tensor("qt", [128, HALF], BF16, kind="ExternalInput")
    kt = nc.dram_tensor("kt", [128, 1024], BF16, kind="ExternalInput")
    # kq0 = [K chunk0 | Q block0] packed for a 1-DMA head
    kq0 = nc.dram_tensor("kq0", [128, 640], BF16, kind="ExternalInput")
    vp = nc.dram_tensor("vp", [128, 8, 65], BF16, kind="ExternalInput")
    out_rect = nc.dram_tensor("out_rect", [4, 128, 4, 65], FP32,
                              kind="ExternalOutput")

    with tile.TileContext(nc) as tc:
        with (
            tc.tile_pool(name="const", bufs=1) as const,
            tc.tile_pool(name="ppool", bufs=3) as ppool,
            tc.tile_pool(name="osb", bufs=2) as osb,
            tc.tile_pool(name="spsum", bufs=2, space="PSUM") as spsum,
            tc.tile_pool(name="opsum", bufs=2, space="PSUM") as opsum,
        ):
            kq0_sb = const.tile([128, 640], BF16, tag="kq0")
            nc.sync.dma_start(out=kq0_sb, in_=kq0[:])
            kt0 = kq0_sb[:, 0:128]
            qt0 = kq0_sb[:, 128:640]
            ktr = const.tile([128, 896], BF16, tag="ktr")
            nc.sync.dma_start(out=ktr, in_=kt[:, 128:1024])
            vp_sb = const.tile([128, 8, 65], BF16, tag="vp")
            nc.sync.dma_start(out=vp_sb, in_=vp[:])
            qtr = const.tile([128, 3, 512], BF16, tag="qtr")
            nc.sync.dma_start(out=qtr, in_=qt[:, 512:2048].rearrange(
                "p (b t) -> p b t", b=3))

            def kt_of(kc):
                if kc == 0:
                    return kt0
                return ktr[:, ts(kc - 1, 128)]

            qt_tiles = [qt0] + [qtr[:, i] for i in range(3)]

            emitter = CausalEmitter(
                nc, lambda kc: vp_sb[:, kc, :], None, out_rect,
                spsum, opsum, ppool, osb, diag=False,
            )
            for qb in range(4):
                emitter.emit_qb(qb, 8, qt_tiles[qb], kt_of)
            emitter.finish()
    _split_sync_waits(nc)
    return nc


_NCS = {}


def get_ncs():
    if not _NCS:
        _NCS["k1"] = build_k1()
        _NCS["k2"] = build_k2()
    return _NCS


def _vp_chunks(v):
    """[n, 64] values -> [128, n/128, 65] bf16 with ones column."""
    n = v.shape[0]
    vvp = np.ones((n, 65), dtype=BF16_NP)
    vvp[:, :64] = v.astype(BF16_NP)
    return np.ascontiguousarray(vvp.reshape(n // 128, 128, 65).transpose(1, 0, 2))


def _pad128(a):
    out = np.zeros((128, a.shape[1]), dtype=BF16_NP)
    out[:64] = a
    return out


def _unpack_o(raw):
    """[4, 128, 4, 65] -> [2048, 65] (row q = qb*512 + qi*128 + p)."""
    return np.asarray(raw).transpose(0, 2, 1, 3).reshape(HALF, 65)


def kernel(x, Wq, Wk, Wv):
    x = np.asarray(x, dtype=np.float32)
    ncs = get_ncs()
    core_ids = list(range(NCORES))

    W3 = np.stack(
        [np.asarray(Wq, np.float32), np.asarray(Wk, np.float32),
         np.asarray(Wv, np.float32)], axis=1,
    ).reshape(D, 192)
    Wb = np.ascontiguousarray(
        W3.reshape(8, 128, 192).transpose(1, 0, 2)
    ).astype(BF16_NP)
    ki = np.arange(128)[:, None]
    qi = np.arange(128)[None, :]
    mask1 = (qi >= ki).astype(BF16_NP)

    in1 = []
    for c in range(NCORES):
        b, hf = divmod(c, 2)
        xs = x[b, hf * HALF : (hf + 1) * HALF, :]
        xt = np.ascontiguousarray(
            xs.reshape(4, 512, 8, 128).transpose(0, 3, 2, 1)
        ).astype(BF16_NP)
        wx0h = np.concatenate(
            [Wb[:, 0], xt[0, :, 0], xt[0, :, 1]], axis=1
        )  # [128, 1216]
        in1.append({"xT": xt, "W": Wb, "mask1": mask1,
                    "wx0": np.ascontiguousarray(wx0h)})
    r1 = run_bass_kernel_spmd(ncs["k1"], in1, core_ids=core_ids).results

    in2 = []
    for c in range(NCORES):
        b, hf = divmod(c, 2)
        # [4, 64, 512] stripes -> [64, 2048]
        qhi = np.asarray(r1[2 * b + 1]["qt_out"]).transpose(1, 0, 2).reshape(64, HALF)
        klo = np.asarray(r1[2 * b]["kt_out"]).transpose(1, 0, 2).reshape(64, HALF)
        vlo = np.asarray(r1[2 * b]["vp_out"])  # [4, 128, 4, 65]
        qtp = _pad128(qhi)
        ktp = _pad128(klo[:, hf * 1024 : (hf + 1) * 1024])
        in2.append(
            {
                "qt": qtp,
                "kt": ktp,
                "kq0": np.ascontiguousarray(
                    np.concatenate([ktp[:, 0:128], qtp[:, 0:512]], axis=1)
                ),
                "vp": np.ascontiguousarray(
                    np.concatenate([vlo[2 * hf], vlo[2 * hf + 1]], axis=1)
                ),
            }
        )
    r2 = run_bass_kernel_spmd(ncs["k2"], in2, core_ids=core_ids).results

    out = np.empty((B, T, DH), dtype=np.float32)
    for b in range(B):
        t0 = _unpack_o(r1[2 * b]["out_tri"])
        out[b, :HALF] = t0[:, :64] / t0[:, 64:65]
        acc = _unpack_o(r1[2 * b + 1]["out_tri"])
        acc = acc + _unpack_o(r2[2 * b]["out_rect"])
        acc = acc + _unpack_o(r2[2 * b + 1]["out_rect"])
        out[b, HALF:] = acc[:, :64] / acc[:, 64:65]
    return out



# revision 24
# speedup vs baseline: 1.1009x; 1.1009x over previous
"""Causal single-head attention (B=4, T=4096, D=1024, D_H=64) on 8 TRN2 cores.

Two SPMD Bass kernels with a host exchange between them. Work split per
batch b over cores (2b, 2b+1); core 2b+h owns rows [h*2048, (h+1)*2048):

Kernel 1 (all core-local):
  - Fused QKV projection: x 128x128 chunks are the stationary operand, the
    concatenated [Wq|Wk|Wv] [128, 192] block streams through, accumulating
    [t,h]-natural Q/K/V in PSUM over the 8 d-chunks.
  - Q/K chunks are PE-transposed to [h, t] layout for the score matmuls;
    V stays natural and gets a ones column appended (the softmax
    denominator then falls out of the AV matmul for free).
  - The local 2048x2048 causal triangle: S^T = K_chunk @ Q^T on PE (keys on
    partitions), exp on ACT (the 1/32 scale is folded into the activation),
    triangular-block mask multiply on DVE for diagonal chunks, then
    O[q,65] += P_chunk^T @ V' with the P chunk stationary (65-wide moving
    operand). Diagonal chunks only compute their causally valid suffix.
  - Exports the natural-layout QKV stripes for kernel 2.

Kernel 2 (after a host reshuffle): the dense rows 2048..4095 x
keys [c%2 *1024 ..+1024) rectangle of batch b. Same S/exp/AV pipeline.

Host combines: rows 0..2047 from core 2b's triangle; rows 2048..4095 =
(tri-bot + rect-left + rect-right numerators)/(summed denominators).
Plain exp without max-subtraction is safe: |scores| <~ 0.5.
"""

import numpy as np
import ml_dtypes

import concourse.bass as bass
import concourse.tile as tile
import concourse.mybir as mybir
from concourse.bass import ts
from concourse.bass_utils import run_bass_kernel_spmd

BF16_NP = ml_dtypes.bfloat16
BF16 = mybir.dt.bfloat16
FP32 = mybir.dt.float32

B, T, D, DH = 4, 4096, 1024, 64
HALF = T // 2
NCORES = 8
SCALE = float(D) ** -0.5  # 1/32, applied inside the exp activation
GROUP = 3                 # S chunks exp'd per ACT instruction (3 PSUM banks)
TRI_NKC = [4, 8, 12, 16]  # 128-key chunks per 512-query block (causal)

# tuning knobs (read by build_k1/build_k2)
CFG = {
    "xb3_early": False,    # prefetch stripe-3 x at the top
    "kt_shift_pool": False,
    "qt_copy_dve": True,
}


# ---------------------------------------------------------------------------
# Workaround: this walrus build rejects instructions carrying more than one
# sync wait ("Too many sync wait commands" in setupSyncWait). Tile's
# add_semaphores stage attaches up to ~3 waits per instruction. Post-pass:
# hoist all but the last wait of every instruction into preceding same-engine
# single-wait NoOps (engines execute their stream in order, so this is
# semantically identical).
# ---------------------------------------------------------------------------
def _split_sync_waits(nc):
    for fn in nc.m.functions:
        for bb in fn.blocks:
            insts = list(bb.instructions)
            out, ctr = [], 0
            for inst in insts:
                si = inst.sync_info
                waits = list(si.on_wait) if (si is not None and si.on_wait) else []
                if len(waits) > 1:
                    for w in waits[:-1]:
                        nop = mybir.InstNoOp(
                            name=f"{inst.name}__swait{ctr}",
                            engine=inst.engine,
                            ins=[],
                            outs=[],
                            sync_info=mybir.SyncInfo(on_wait=[w], on_update=[]),
                        )
                        out.append(nop)
                        ctr += 1
                    inst.sync_info = mybir.SyncInfo(
                        on_wait=[waits[-1]],
                        on_update=list(si.on_update or []),
                    )
                out.append(inst)
            if ctr:
                bb.instructions = out


def _tri_groups(nkc, pattern=None):
    """(kc0, gsz) pairs; pattern overrides the default 3-3-... split."""
    if pattern is not None:
        assert sum(pattern) == nkc, (pattern, nkc)
        out, kc0 = [], 0
        for g in pattern:
            out.append((kc0, g))
            kc0 += g
        return out
    out, kc0 = [], 0
    while kc0 < nkc:
        g = min(GROUP, nkc - kc0)
        out.append((kc0, g))
        kc0 += g
    return out


class CausalEmitter:
    """Emits S -> exp -> (mask) -> AV chunk groups, the AV matmuls lagging one
    exp group behind so PE always has S work queued while ACT runs.

    S^T [k=128, q<=512] is computed with K^T chunks stationary; AV uses the
    P chunk [k=128, q=128] as the stationary operand and V' [128, 65]
    moving, accumulating O[q, 65] naturally per 128-query sub-block into a
    single-bank [128, 4, 65] PSUM accumulator.

    If diag, the last 4 chunks of query block qb are diagonal: chunk with
    diag offset j covers keys [128j, 128j+128) relative to the block, so
    only the column suffix [128j:512] is computed, sub-blocks qi < j are
    skipped, and the triangular 128x128 head block gets a mask multiply.
    """

    def __init__(self, nc, vp_of, mask_sb, out_dram, spsum, opsum, ppool,
                 osb, diag, auto_flush=True):
        self.nc = nc
        self.vp_of = vp_of
        self.mask_sb = mask_sb
        self.out_dram = out_dram
        self.spsum = spsum
        self.opsum = opsum
        self.ppool = ppool
        self.osb = osb
        self.diag = diag
        self.auto_flush = auto_flush
        self.pend = []

    def _flush_one(self):
        nc = self.nc
        o_ps, p_sb, entries, nkc, qb = self.pend.pop(0)
        done = False
        for i, kc, off in entries:
            qi_min = off // 128
            for qi in range(qi_min, 4):
                if self.diag:
                    stop = kc == nkc - 4 + qi
                else:
                    stop = kc == nkc - 1
                # start=True clears the has_written bits of the WHOLE bank,
                # so only the very first AV matmul of this accumulator may
                # set it; later sub-blocks overwrite via their cleared bits.
                nc.tensor.matmul(
                    o_ps[:, qi, :],
                    lhsT=p_sb[:, i, ts(qi, 128)],
                    rhs=self.vp_of(kc),
                    start=(kc == 0 and qi == qi_min),
                    stop=stop,
                )
            done = kc == nkc - 1
        if done:
            o_sb = self.osb.tile([128, 4, 65], FP32, tag="o_sb")
            nc.vector.tensor_copy(out=o_sb, in_=o_ps)
            nc.sync.dma_start(out=self.out_dram[qb], in_=o_sb)

    def emit_qb_gen(self, qb, nkc, qt_tile, kt_of, pattern=None, banks=GROUP,
                    dve_groups=()):
        """Generator form: yields after each S/exp group so the caller can
        interleave other PE work (the next stripe's projection) mid-block.

        Groups whose index is in dve_groups compute exp on DVE instead of
        ACT, via the monic Taylor-4 polynomial in the raw score s:
          24*32^4*exp(s/32) ~ (((s+128)s + 12288)s + 786432)s + 25165824
        (relative error < 1e-3 for |s/32| < 0.6). The final tensor_scalar
        folds the 1/(24*32^4) normalization so ACT- and DVE-computed groups
        stay on the same scale.
        """
        nc = self.nc
        ADD, MUL = mybir.AluOpType.add, mybir.AluOpType.mult
        o_ps = self.opsum.tile([128, 4, 65], FP32, tag="o_ps")
        for gi, (kc0, gsz) in enumerate(_tri_groups(nkc, pattern)):
            s_ps = self.spsum.tile([128, banks, 512], FP32, tag="s_ps")
            entries = []
            for i in range(gsz):
                kc = kc0 + i
                j = kc - (nkc - 4) if self.diag else -1
                off = 128 * j if j > 0 else 0
                entries.append((i, kc, off))
                # 64-wide contraction: K^T/Q^T live in partitions 0..63; no
                # zero-padding of rows 64..127 needed.
                nc.tensor.matmul(
                    s_ps[:, i, off:],
                    lhsT=kt_of(kc),
                    rhs=qt_tile[0:64, off:512],
                    start=True,
                    stop=True,
                )
            p_sb = self.ppool.tile([128, banks, 512], BF16, tag="p_sb")
            if gi in dve_groups:
                s_sl = s_ps[:, :gsz, :]
                tmp = self.ppool.tile([128, banks, 512], FP32, tag="poly")
                t_sl = tmp[:, :gsz, :]
                nc.vector.scalar_tensor_tensor(t_sl, s_sl, 128.0, s_sl,
                                               ADD, MUL)
                nc.vector.scalar_tensor_tensor(t_sl, t_sl, 12288.0, s_sl,
                                               ADD, MUL)
                nc.vector.scalar_tensor_tensor(t_sl, t_sl, 786432.0, s_sl,
                                               ADD, MUL)
                nc.vector.tensor_scalar(p_sb[:, :gsz, :], t_sl, 25165824.0,
                                        1.0 / 25165824.0, ADD, MUL)
            else:
                nc.scalar.activation(
                    out=p_sb[:, :gsz, :],
                    in_=s_ps[:, :gsz, :],
                    func=mybir.ActivationFunctionType.Exp,
                    scale=SCALE,
                )
            if self.diag:
                for i, kc, off in entries:
                    j = kc - (nkc - 4)
                    if j >= 0:
                        moff = 128 * j
                        nc.vector.tensor_mul(
                            out=p_sb[:, i, moff : moff + 128],
                            in0=p_sb[:, i, moff : moff + 128],
                            in1=self.mask_sb,
                        )
            self.pend.append((o_ps, p_sb, entries, nkc, qb))
            if self.auto_flush and len(self.pend) > 1:
                self._flush_one()
            yield

    def emit_qb(self, qb, nkc, qt_tile, kt_of, pattern=None, banks=GROUP,
                dve_groups=()):
        for _ in self.emit_qb_gen(qb, nkc, qt_tile, kt_of, pattern, banks,
                                  dve_groups):
            pass

    def finish(self):
        while self.pend:
            self._flush_one()


# ---------------------------------------------------------------------------
# Kernel 1: fused QKV projection + local causal triangle.
# ---------------------------------------------------------------------------
K1_PATTERNS = [[1, 2, 1], [2, 2, 2, 2], [2] * 6, [2] * 7 + [1, 1]]
# after which S/exp group of tri(t) to inject proj(t+1)
K1_INJECT = [1, 2, 2, None]
K1_DVE = [(), (), (), ()]     # exp groups computed on DVE (poly)
K1_BANKS = 2  # S-group PSUM tiles are 2 banks; proj pool takes the other 2


def build_k1():
    nc = bass.Bass()
    # xT[tb, p, dc, t] = x_shard[tb*512 + t, dc*128 + p]
    xT = nc.dram_tensor("xT", [4, 128, 8, 512], BF16, kind="ExternalInput")
    # w01 = [W chunk0 | W chunk1] (tiny head DMA so ldweights starts early)
    w01 = nc.dram_tensor("w01", [128, 384], BF16, kind="ExternalInput")
    # wrm = [W chunks 2..7 | mask1] packed (mask1[k, q] = 1.0 if q >= k)
    wrm = nc.dram_tensor("wrm", [128, 1280], BF16, kind="ExternalInput")
    # exports: per stripe t, raw [Q^T; K^T] [128, 512] and V' [128, 4, 65]
    qkt_out = nc.dram_tensor("qkt_out", [128, 4, 512], BF16,
                             kind="ExternalOutput")
    vp_out = nc.dram_tensor("vp_out", [128, 4, 4, 65], BF16,
                            kind="ExternalOutput")
    # out_tri[qb, p, qi, m]: numerator cols 0..63, denominator col 64 for
    # query row qb*512 + qi*128 + p
    out_tri = nc.dram_tensor("out_tri", [4, 128, 4, 65], FP32,
                             kind="ExternalOutput")

    with tile.TileContext(nc) as tc:
        with (
            tc.tile_pool(name="const", bufs=1) as const,
            tc.tile_pool(name="xpool", bufs=3) as xpool,
            tc.tile_pool(name="ppool", bufs=7) as ppool,
            tc.tile_pool(name="osb", bufs=2) as osb,
            tc.tile_pool(name="spsum", bufs=2, space="PSUM") as spsum,
            tc.tile_pool(name="projsum", bufs=1, space="PSUM") as projsum,
            tc.tile_pool(name="opsum", bufs=2, space="PSUM") as opsum,
        ):

            w01_sb = const.tile([128, 384], BF16, tag="w01")
            nc.sync.dma_start(out=w01_sb, in_=w01[:])
            wrm_sb = const.tile([128, 1280], BF16, tag="wrm")
            mask_sb = wrm_sb[:, 1152:1280]

            qt_all = const.tile([128, 4, 512], BF16, tag="qt_all")
            vp_all = const.tile([128, 4, 4, 65], BF16, tag="vp_all")
            kt_t = []
            emitter = CausalEmitter(
                nc,
                lambda kc: vp_all[:, kc // 4, kc % 4, :],
                mask_sb, out_tri, spsum, opsum, ppool, osb, diag=True,
                auto_flush=True,
            )

            # x prefetch: stripe 0 in 2-d-chunk pieces (fast head), stripes
            # 1-2 issued immediately behind it, stripe 3 later.
            x0_t = []
            for i in range(2):
                xp = xpool.tile([128, 1, 512], BF16, tag=f"x0s_{i}")
                nc.sync.dma_start(out=xp, in_=xT[0, :, i : i + 1])
                x0_t.append(xp[:, 0])
            for i in range(1, 4):
                xp = xpool.tile([128, 2, 512], BF16, tag=f"x0_{i}")
                nc.sync.dma_start(out=xp, in_=xT[0, :, ts(i, 2)])
                if i == 1:
                    nc.sync.dma_start(out=wrm_sb, in_=wrm[:])
                x0_t.extend([xp[:, 0], xp[:, 1]])
            w_t = [w01_sb[:, 0:192], w01_sb[:, 192:384]] + [
                wrm_sb[:, dc * 192 - 384 : dc * 192 - 192] for dc in range(2, 8)]
            xbig = {}
            for tt in (1, 2):
                xs = xpool.tile([128, 8, 512], BF16, tag="xbig")
                nc.sync.dma_start(out=xs, in_=xT[tt])
                xbig[tt] = xs

            def proj(t):
                """Project stripe t: merged QK into a 2-bank proj slot, V
                natural, then the copies/realign. For t>0 this is emitted
                MID-qb(t-1) so PE fills exp-pacing stalls with it."""
                if t > 0 and t not in xbig:
                    xs = xpool.tile([128, 8, 512], BF16, tag="xbig")
                    nc.sync.dma_start(out=xs, in_=xT[t])
                    xbig[t] = xs
                x_t = x0_t if t == 0 else [xbig[t][:, dc] for dc in range(8)]

                # qt holds [Q^T; K^T] raw from the merged projection; the S
                # matmuls contract only partitions 0..63 so no zeroing is
                # needed. kt gets K^T realigned to partitions 0..63 via an
                # SBUF->SBUF DMA (engines cannot shift partitions; DMA can).
                qt = qt_all[:, t, :]
                kt = const.tile([128, 512], BF16, tag=f"k{t}")
                vp = vp_all[:, t]
                nc.vector.memset(vp[:, :, 64:65], 1.0)
                kt_t.append(kt)

                slot = projsum.tile([128, 2, 512], FP32, tag="proj")
                for dc in range(8):
                    nc.tensor.matmul(
                        slot[:, 0, :], lhsT=w_t[dc][:, 0:128],
                        rhs=x_t[dc], start=(dc == 0), stop=(dc == 7),
                    )
                nc.vector.tensor_copy(out=qt, in_=slot[:, 0, :])
                if t == 0:
                    # stripe 0's first S is on the kernel head's critical
                    # path: project K directly (PE is DMA-stalled anyway)
                    # instead of waiting for copy -> realign-DMA
                    kslot = spsum.tile([128, K1_BANKS, 512], FP32, tag="s_ps")
                    for dc in range(8):
                        nc.tensor.matmul(
                            kslot[:64, 0, :], lhsT=w_t[dc][:, 64:128],
                            rhs=x_t[dc], start=(dc == 0), stop=(dc == 7),
                        )
                    nc.scalar.copy(out=kt[0:64, :], in_=kslot[:64, 0, :])
                else:
                    # K^T realign to partitions 0..63 -- hides behind the
                    # remaining S/exp groups of qb(t-1)
                    nc.sync.dma_start(out=kt[0:64, :], in_=qt_all[64:128, t, :])
                for dc in range(8):
                    for tc_ in range(4):
                        nc.tensor.matmul(
                            slot[:, 1, ts(tc_, 64)],
                            lhsT=x_t[dc][:, ts(tc_, 128)],
                            rhs=w_t[dc][:, 128:192],
                            start=(dc == 0 and tc_ == 0),
                            stop=(dc == 7 and tc_ == 3),
                        )
                nc.vector.tensor_copy(
                    out=vp[:, :, 0:64],
                    in_=slot[:, 1, 0:256].rearrange("p (c h) -> p c h", c=4),
                )
                if t == 3:
                    # single batched export of all stripes' raw QKT + V'
                    # (k2 inputs); off the critical path, one DMA each.
                    nc.sync.dma_start(out=qkt_out[:], in_=qt_all)
                    nc.sync.dma_start(out=vp_out[:], in_=vp_all)

            # Software pipeline: stripe t's S/exp/AV groups flow while the
            # NEXT stripe's projection is injected mid-block, keeping PE busy
            # during the exp-paced sections and hiding the copy/realign chain.
            proj(0)
            for t in range(4):
                gen = emitter.emit_qb_gen(
                    t, TRI_NKC[t], qt_all[:, t, :],
                    lambda kc: kt_t[kc // 4][0:64, ts(kc % 4, 128)],
                    pattern=K1_PATTERNS[t], banks=K1_BANKS,
                    dve_groups=K1_DVE[t],
                )
                g = 0
                for _ in gen:
                    g += 1
                    if t < 3 and g == K1_INJECT[t]:
                        proj(t + 1)
            emitter.finish()
    _split_sync_waits(nc)
    return nc


# ---------------------------------------------------------------------------
# Kernel 2: dense 2048q x 1024k rectangle.
# ---------------------------------------------------------------------------
K2_PATTERNS = [[1, 3, 3, 1], [3, 3, 2], [3, 3, 2], [3, 3, 1, 1]]
K2_DVE = [(), (), (), ()]


def build_k2():
    nc = bass.Bass()
    # all Q^T/K^T inputs are 64 valid rows only (no 128-pad)
    qt = nc.dram_tensor("qt", [64, HALF], BF16, kind="ExternalInput")
    kt = nc.dram_tensor("kt", [64, 1024], BF16, kind="ExternalInput")
    # kq0 = [K chunk0 | Q block0] packed for a 1-DMA head
    kq0 = nc.dram_tensor("kq0", [64, 640], BF16, kind="ExternalInput")
    vp = nc.dram_tensor("vp", [128, 8, 65], BF16, kind="ExternalInput")
    out_rect = nc.dram_tensor("out_rect", [4, 128, 4, 65], FP32,
                              kind="ExternalOutput")

    with tile.TileContext(nc) as tc:
        with (
            tc.tile_pool(name="const", bufs=1) as const,
            tc.tile_pool(name="ppool", bufs=3) as ppool,
            tc.tile_pool(name="osb", bufs=2) as osb,
            tc.tile_pool(name="spsum", bufs=2, space="PSUM") as spsum,
            tc.tile_pool(name="opsum", bufs=2, space="PSUM") as opsum,
        ):
            kq0_sb = const.tile([128, 640], BF16, tag="kq0")
            nc.sync.dma_start(out=kq0_sb[0:64, :], in_=kq0[:])
            ktr = const.tile([128, 896], BF16, tag="ktr")
            nc.sync.dma_start(out=ktr[0:64, :], in_=kt[:, 128:1024])
            vp_sb = const.tile([128, 8, 65], BF16, tag="vp")
            nc.sync.dma_start(out=vp_sb, in_=vp[:])
            qtr = const.tile([128, 3, 512], BF16, tag="qtr")
            nc.sync.dma_start(out=qtr[0:64, :, :], in_=qt[:, 512:2048].rearrange(
                "p (b t) -> p b t", b=3))

            def kt_of(kc):
                if kc == 0:
                    return kq0_sb[0:64, 0:128]
                return ktr[0:64, ts(kc - 1, 128)]

            qt_tiles = [kq0_sb[:, 128:640]] + [qtr[:, i] for i in range(3)]

            emitter = CausalEmitter(
                nc, lambda kc: vp_sb[:, kc, :], None, out_rect,
                spsum, opsum, ppool, osb, diag=False,
            )
            for qb in range(4):
                emitter.emit_qb(qb, 8, qt_tiles[qb], kt_of,
                                pattern=K2_PATTERNS[qb],
                                dve_groups=K2_DVE[qb])
            emitter.finish()
    _split_sync_waits(nc)
    return nc


_NCS = {}


def get_ncs():
    if not _NCS:
        _NCS["k1"] = build_k1()
        _NCS["k2"] = build_k2()
    return _NCS


def _vp_chunks(v):
    """[n, 64] values -> [128, n/128, 65] bf16 with ones column."""
    n = v.shape[0]
    vvp = np.ones((n, 65), dtype=BF16_NP)
    vvp[:, :64] = v.astype(BF16_NP)
    return np.ascontiguousarray(vvp.reshape(n // 128, 128, 65).transpose(1, 0, 2))


def _unpack_o(raw):
    """[4, 128, 4, 65] -> [2048, 65] (row q = qb*512 + qi*128 + p)."""
    return np.asarray(raw).transpose(0, 2, 1, 3).reshape(HALF, 65)


def kernel(x, Wq, Wk, Wv):
    x = np.asarray(x, dtype=np.float32)
    ncs = get_ncs()
    core_ids = list(range(NCORES))

    W3 = np.stack(
        [np.asarray(Wq, np.float32), np.asarray(Wk, np.float32),
         np.asarray(Wv, np.float32)], axis=1,
    ).reshape(D, 192)
    Wb = np.ascontiguousarray(
        W3.reshape(8, 128, 192).transpose(1, 0, 2)
    ).astype(BF16_NP)
    ki = np.arange(128)[:, None]
    qi = np.arange(128)[None, :]
    mask1 = (qi >= ki).astype(BF16_NP)
    wrm = np.ascontiguousarray(
        np.concatenate([Wb[:, 2:8].reshape(128, 1152), mask1], axis=1)
    )  # [128, 1280]
    w01h = np.ascontiguousarray(Wb[:, 0:2].reshape(128, 384))

    in1 = []
    for c in range(NCORES):
        b, hf = divmod(c, 2)
        xs = x[b, hf * HALF : (hf + 1) * HALF, :]
        xt = np.ascontiguousarray(
            xs.reshape(4, 512, 8, 128).transpose(0, 3, 2, 1)
        ).astype(BF16_NP)
        in1.append({"xT": xt, "wrm": wrm, "w01": w01h})
    r1 = run_bass_kernel_spmd(ncs["k1"], in1, core_ids=core_ids).results

    in2 = []
    for c in range(NCORES):
        b, hf = divmod(c, 2)
        # qkt_out [128, 4, 512]: rows 0..63 = Q^T stripes, 64..127 = K^T
        qhi = np.asarray(r1[2 * b + 1]["qkt_out"])[0:64].reshape(64, HALF)
        klo = np.asarray(r1[2 * b]["qkt_out"])[64:128].reshape(64, HALF)
        vlo = np.asarray(r1[2 * b]["vp_out"])  # [128, 4, 4, 65]
        ktp = np.ascontiguousarray(klo[:, hf * 1024 : (hf + 1) * 1024])
        in2.append(
            {
                "qt": np.ascontiguousarray(qhi),
                "kt": ktp,
                "kq0": np.ascontiguousarray(
                    np.concatenate([ktp[:, 0:128], qhi[:, 0:512]], axis=1)
                ),
                "vp": np.ascontiguousarray(
                    vlo[:, 2 * hf : 2 * hf + 2].reshape(128, 8, 65)
                ),
            }
        )
    r2 = run_bass_kernel_spmd(ncs["k2"], in2, core_ids=core_ids).results

    out = np.empty((B, T, DH), dtype=np.float32)
    for b in range(B):
        t0 = _unpack_o(r1[2 * b]["out_tri"])
        out[b, :HALF] = t0[:, :64] / t0[:, 64:65]
        acc = _unpack_o(r1[2 * b + 1]["out_tri"])
        acc = acc + _unpack_o(r2[2 * b]["out_rect"])
        acc = acc + _unpack_o(r2[2 * b + 1]["out_rect"])
        out[b, HALF:] = acc[:, :64] / acc[:, 64:65]
    return out

